# revision 1
# baseline (speedup 1.0000x reference)
"""DMNN (dendritic memory NN) forward kernel for Trainium2, 8-core data-parallel.

Math (per batch row x of inp [B, D]):
    sq[ck]   = ||x||^2 + ||c_ck||^2 - 2 x.c_ck        (ck = (c, k), C=2 classes x K=512 dendrites)
    t[ck]    = sqrt(sq + eps)
    d[ck]    = radii[ck] - t[ck]
    per class c:  S_c = sum_k exp(d),  T_oc = sum_k W[o,c,k] * d * exp(d)
    logits_o = sum_c T_oc / S_c + sum_c b[o,c]
    out      = softmax(logits)  ==  sigmoid(+/-(l1 - l0 + db))

Device mapping (per core, B_c = 8192 rows):
  - Layout: dendrites-on-partitions, batch-on-free. Host supplies inp
    transposed+augmented: xin [66, B_c] = [inp.T; ||x||^2; ones].
  - sq comes straight out of the PE via an augmented K=66 matmul with
    lhsT = [-2 c.T; ones; ||c||^2 + eps] (float32r for full-rate fp32).
  - ACT does the two transcendental passes (sqrt from PSUM, exp from SBUF);
    exp(radii) is folded into the reduction weights host-side so the exp op
    needs no per-tile bias (big free dims, fewer table switches).
  - S/T reductions over k are K=128 PE matmuls (rhs = f / t*f tiles), with
    tile_position column tiling so 4 batch-tiles' reductions run concurrently.
  - sqrt and exp live in different ACT table sets (~2.7us/switch), so work is
    phased in quads of 4 batch-tiles: all sqrts, then all exps.
  - Tail (per-class normalization + 2-way softmax) runs on relaid [128, 64]
    stat tiles; final probs are interleaved on-chip and stored contiguously.
"""

import os
import sys

os.environ.setdefault("MYCRO_LOCAL_CACHE", "1")
if "/opt/trn_rl_repo" not in sys.path:
    sys.path.insert(0, "/opt/trn_rl_repo")

from contextlib import ExitStack

import numpy as np

import concourse.bacc as bacc
import concourse.tile as tile
from concourse import mybir
from concourse.bass_utils import run_bass_kernel_spmd
from concourse.tile import add_dep_helper

B, D, C, K = 65536, 2, 512, 64  # noqa: E741  (names per reference: B batch, C classes, K dendrites, D dim)
B, DIM, NCLS, NDEN = 65536, 64, 2, 512
CK = NCLS * NDEN            # 1024 dendrites total
NCORES = 8
BC = B // NCORES            # 8192 batch rows per core
NBT = 512                   # batch columns per tile (fp32 PSUM bank width)
NT = BC // NBT              # 16 batch tiles per core
QUAD = 4                    # batch tiles per ACT table phase (and per stats bank)
NQ = NT // QUAD             # 4 quads
CKT = CK // 128             # 8 dendrite tiles of 128
KAUG = DIM + 2              # 66: contraction with x2 and c2 rows folded in
SQ_EPS = 1e-6

F32 = mybir.dt.float32
F32R = mybir.dt.float32r
AF = mybir.ActivationFunctionType

_CACHED_NC = None


def _build_module(loops=1):
    nc = bacc.Bacc(
        "TRN2",
        target_bir_lowering=False,
        debug=False,
        enable_asserts=False,
        num_devices=NCORES,
    )
    xin_d = nc.dram_tensor("xin", [KAUG, BC], F32, kind="ExternalInput").ap()
    clhs_d = nc.dram_tensor("clhs", [KAUG, CK], F32, kind="ExternalInput").ap()
    elhs_d = nc.dram_tensor("elhs", [128, CKT * 32], F32, kind="ExternalInput").ap()
    tlhs_d = nc.dram_tensor("tlhs", [128, CKT * 32], F32, kind="ExternalInput").ap()
    sgb_d = nc.dram_tensor("sgb", [128, 2], F32, kind="ExternalInput").ap()
    out_d = nc.dram_tensor("out", [BC, 2], F32, kind="ExternalOutput").ap()

    with tile.TileContext(nc) as tc:
        _kernel_body(tc, out_d, xin_d, clhs_d, elhs_d, tlhs_d, sgb_d, loops)
    nc.compile()
    return nc


def _kernel_body(tc, out_d, xin_d, clhs_d, elhs_d, tlhs_d, sgb_d, loops=1):
    nc = tc.nc
    with ExitStack() as ctx:
        if loops > 1:
            # hardware loop for benchmarking: repeats the whole computation
            ctx.enter_context(tc.For_i(
                0, loops, 1,
                hint_engines=(mybir.EngineType.PE, mybir.EngineType.Activation,
                              mybir.EngineType.DVE, mybir.EngineType.SP),
            ))
        persist = ctx.enter_context(tc.tile_pool(name="persist", bufs=1))
        tpool = ctx.enter_context(tc.tile_pool(name="tpool", bufs=QUAD))
        fpool = ctx.enter_context(tc.tile_pool(name="fpool", bufs=3))
        gpool = ctx.enter_context(tc.tile_pool(name="gpool", bufs=3))
        stage = ctx.enter_context(tc.tile_pool(name="stage", bufs=4))
        drbp = ctx.enter_context(tc.tile_pool(name="drbp", bufs=4, space="DRAM"))
        sqpool = ctx.enter_context(tc.tile_pool(name="sqpool", bufs=3, space="PSUM"))
        stpool = ctx.enter_context(tc.tile_pool(name="stpool", bufs=2, space="PSUM"))

        # ---- persistent inputs ----
        # walrus requires float32r matmul operands to come from a compute op
        # ("rounded to FP32r"), so inputs bounce through small fp32 tiles and
        # a DVE copy produces the fp32r-typed SBUF residents.
        bounce = ctx.enter_context(tc.tile_pool(name="bounce", bufs=2))
        # params first: the very first dots matmul needs clhs, so it must not
        # queue behind the bulk xin transfer.
        clhs = persist.tile([KAUG, CK], F32R, tag="clhs")
        bc1 = bounce.tile([KAUG, CK], F32, tag="bc1", name="bc1")
        nc.sync.dma_start(bc1[:], clhs_d[:])
        nc.vector.tensor_copy(clhs[:], bc1[:])
        elhs = persist.tile([128, CKT * 32], F32R, tag="elhs")
        bc2 = bounce.tile([128, CKT * 32], F32, tag="bc2", name="bc2")
        nc.sync.dma_start(bc2[:], elhs_d[:])
        nc.vector.tensor_copy(elhs[:], bc2[:])
        tlhs = persist.tile([128, CKT * 32], F32R, tag="tlhs")
        bc3 = bounce.tile([128, CKT * 32], F32, tag="bc3", name="bc3")
        nc.sync.dma_start(bc3[:], tlhs_d[:])
        nc.vector.tensor_copy(tlhs[:], bc3[:])
        sgb = persist.tile([128, 2], F32, tag="sgb")
        nc.sync.dma_start(sgb[:], sgb_d[:])
        xrpool = ctx.enter_context(tc.tile_pool(name="xrpool", bufs=3))
        xbpool = ctx.enter_context(tc.tile_pool(name="xbpool", bufs=3))

        # relaid stats, one tile: statAll[p, s*64 + f] = stat s of batch row
        # b = p*64 + f.  stat order: 0=S0 1=T00 2=T10 3=S1 4=T01 5=T11
        statAll = persist.tile([128, 6 * 64], F32, tag="statAll")

        # ACT-engine phase ordering: the scheduler would otherwise interleave
        # sqrt/exp ops across quads, paying a ~2.7us table switch each time.
        # Dots matmuls are emitted with a 3-tile lookahead so the PE fills sq
        # PSUM tiles for quad q+1 while quad q's phase B is still running --
        # the first sqrt of q+1 then starts the moment ACT swaps tables.
        last_exp_inst = None
        last_sqrt_inst = None
        relayout_dmas = []
        entries = [(jj, pair) for jj in range(NT) for pair in range(CKT // 2)]
        sq_fifo = []
        emit_state = {"idx": 0}

        xr_cur = {}

        def emit_next_dots():
            jj, pair = entries[emit_state["idx"]]
            emit_state["idx"] += 1
            if pair == 0:
                bx = xbpool.tile([KAUG, NBT], F32, tag="bx", name="bx")
                nc.sync.dma_start(bx[:], xin_d[:, jj * NBT:(jj + 1) * NBT])
                xr = xrpool.tile([KAUG, NBT], F32R, tag="xr", name="xr")
                nc.vector.tensor_copy(xr[:], bx[:])
                xr_cur[0] = xr
            rhs = xr_cur[0][:]
            sq = sqpool.tile([128, 2 * NBT], F32, tag="sq", name="sq")
            for h in range(2):
                t_ck = pair * 2 + h
                nc.tensor.matmul(
                    sq[:, h * NBT:(h + 1) * NBT],
                    clhs[:, t_ck * 128:(t_ck + 1) * 128],
                    rhs,
                    start=True,
                    stop=True,
                )
            sq_fifo.append(sq)

        for _ in range(3):
            emit_next_dots()
        for q in range(NQ):
            # ---------- phase A (sqrt table): sqrt of pipelined sq tiles ----------
            ttiles = []
            prev_exp = last_exp_inst
            for j in range(QUAD):
                tt = tpool.tile([128, CKT * NBT], F32R, tag="t", name="tt")
                ttiles.append(tt)
                for pair in range(CKT // 2):
                    sq = sq_fifo.pop(0)
                    last_sqrt_inst = nc.scalar.activation(
                        tt[:, pair * 2 * NBT:(pair + 1) * 2 * NBT], sq[:], AF.Sqrt
                    )
                    if prev_exp is not None:
                        add_dep_helper(last_sqrt_inst.ins, prev_exp.ins, sync=False,
                                       reason="ACT table phase order")
                    if emit_state["idx"] < len(entries):
                        emit_next_dots()

            # ---------- phase B (exp table): f = exp(-t), g = t*f, reductions ----------
            prev_sqrt = last_sqrt_inst
            for j in range(QUAD):
                jj = q * QUAD + j
                tt = ttiles[j]
                ff = fpool.tile([128, CKT * NBT], F32R, tag="f", name="ff")
                last_exp_inst = nc.scalar.activation(ff[:], tt[:], AF.Exp, scale=-1.0)
                add_dep_helper(last_exp_inst.ins, prev_sqrt.ins, sync=False,
                               reason="ACT table phase order")
                gg = gpool.tile([128, CKT * NBT], F32R, tag="g", name="gg")
                half = CKT * NBT // 2
                nc.vector.tensor_mul(gg[:, :half], tt[:, :half], ff[:, :half])
                nc.vector.tensor_mul(gg[:, half:], tt[:, half:], ff[:, half:])
                stats = stpool.tile([32, NBT], F32, tag="stats", name="stats")
                # all e-matmuls first: they only need ff, so ff's pool slot
                # frees right after its exp and the ACT never stalls on it
                for t_ck in range(CKT):
                    nc.tensor.matmul(
                        stats[:],
                        elhs[:, t_ck * 32:(t_ck + 1) * 32],
                        ff[:, t_ck * NBT:(t_ck + 1) * NBT],
                        start=(t_ck == 0),
                        stop=False,
                    )
                for t_ck in range(CKT):
                    nc.tensor.matmul(
                        stats[:],
                        tlhs[:, t_ck * 32:(t_ck + 1) * 32],
                        gg[:, t_ck * NBT:(t_ck + 1) * NBT],
                        start=False,
                        stop=(t_ck == CKT - 1),
                    )
                # ---------- evacuate + relayout this b-tile's stats ----------
                # PSUM -> SBUF (DVE), then through a DRAM bounce: DRAM APs are
                # linear, so the partition-crossing reshuffle is legal on both
                # DMA hops (SBUF APs need the partition dim outermost).
                stg = stage.tile([32, NBT], F32, tag="stg", name="stg")
                cp = nc.vector.tensor_copy(stg[:], stats[:])
                drb = drbp.tile([6, NBT], F32, tag="drb", name="drb")
                dma1 = nc.sync.dma_start(drb[:], stg[0:6, :])
                add_dep_helper(dma1.ins, cp.ins, sync=True,
                               reason="stats relayout reads staged copy")
                # statAll[jj*8 + (b>>6), s*64 + (b&63)] = stat s of b
                dst = statAll[jj * 8:(jj + 1) * 8, :].rearrange(
                    "p (s f) -> p s f", f=64)
                srcv = drb.rearrange("s (p f) -> p s f", f=64)
                dma = nc.sync.dma_start(dst, srcv)
                add_dep_helper(dma.ins, dma1.ins, sync=True,
                               reason="relayout reads dram bounce")
                relayout_dmas.append(dma)

        # ---------- tail: logits + 2-way softmax ----------
        tailp = ctx.enter_context(tc.tile_pool(name="tailp", bufs=1))
        r0 = tailp.tile([128, 64], F32, tag="r0")
        r1 = tailp.tile([128, 64], F32, tag="r1")
        u0 = tailp.tile([128, 64], F32, tag="u0")
        u1 = tailp.tile([128, 64], F32, tag="u1")
        dl = tailp.tile([128, 64], F32, tag="dl")
        p0 = tailp.tile([128, 64], F32, tag="p0")
        p1 = tailp.tile([128, 64], F32, tag="p1")
        outT = tailp.tile([128, 128], F32, tag="outT")

        S0, T00, T10 = statAll[:, 0:64], statAll[:, 64:128], statAll[:, 128:192]
        S1, T01, T11 = statAll[:, 192:256], statAll[:, 256:320], statAll[:, 320:384]
        rc0 = nc.vector.reciprocal(r0[:], S0)
        for d in relayout_dmas:
            add_dep_helper(rc0.ins, d.ins, sync=True,
                           reason="tail reads relaid stats")
        nc.vector.reciprocal(r1[:], S1)
        nc.vector.tensor_sub(u0[:], T10, T00)
        nc.vector.tensor_sub(u1[:], T11, T01)
        nc.vector.tensor_mul(u0[:], u0[:], r0[:])
        nc.vector.tensor_mul(u1[:], u1[:], r1[:])
        nc.vector.tensor_add(dl[:], u0[:], u1[:])                # l1 - l0
        sig1 = nc.scalar.activation(p1[:], dl[:], AF.Sigmoid, bias=sgb[:, 0:1], scale=1.0)
        add_dep_helper(sig1.ins, last_exp_inst.ins, sync=False,
                       reason="ACT table phase order")
        nc.scalar.activation(p0[:], dl[:], AF.Sigmoid, bias=sgb[:, 1:2], scale=-1.0)
        outT_r = outT.rearrange("p (f c) -> p f c", c=2)
        nc.vector.tensor_copy(outT_r[:, :, 0], p0[:])
        nc.vector.tensor_copy(outT_r[:, :, 1], p1[:])
        nc.sync.dma_start(out_d.rearrange("(p f) c -> p (f c)", p=128), outT[:])


def _prep_inputs(inp, centroids, radii, W, b):
    inp = np.ascontiguousarray(np.asarray(inp, dtype=np.float32))
    cents = np.asarray(centroids, dtype=np.float32)
    radii = np.asarray(radii, dtype=np.float32)
    W = np.asarray(W, dtype=np.float32)
    b = np.asarray(b, dtype=np.float32)

    x2 = np.einsum("bd,bd->b", inp, inp, dtype=np.float32)
    xin = np.empty((KAUG, B), np.float32)
    xin[:DIM] = inp.T
    xin[DIM] = x2
    xin[DIM + 1] = 1.0

    cT = cents.reshape(CK, DIM)                       # [1024, 64], ck = c*512 + k
    c2 = np.einsum("cd,cd->c", cT, cT, dtype=np.float32)
    clhs = np.empty((KAUG, CK), np.float32)
    clhs[:DIM] = -2.0 * cT.T
    clhs[DIM] = 1.0
    clhs[DIM + 1] = c2 + SQ_EPS

    rflat = radii.reshape(CK)
    eflat = np.exp(rflat)
    Wf = W.reshape(2, CK)                             # [o, c*512+k]
    elhs = np.zeros((128, CKT * 32), np.float32)
    tlhs = np.zeros((128, CKT * 32), np.float32)
    for t in range(CKT):
        ckr = slice(t * 128, (t + 1) * 128)
        c = t // (CKT // NCLS)
        ew = eflat[ckr]
        elhs[:, t * 32 + 3 * c + 0] = ew
        elhs[:, t * 32 + 3 * c + 1] = Wf[0, ckr] * rflat[ckr] * ew
        elhs[:, t * 32 + 3 * c + 2] = Wf[1, ckr] * rflat[ckr] * ew
        tlhs[:, t * 32 + 3 * c + 1] = -Wf[0, ckr] * ew
        tlhs[:, t * 32 + 3 * c + 2] = -Wf[1, ckr] * ew

    bs = b.sum(axis=1)                                # [2]
    db = np.float32(bs[1] - bs[0])
    sgb = np.zeros((128, 2), np.float32)
    sgb[:, 0] = db
    sgb[:, 1] = -db

    in_maps = []
    for m in range(NCORES):
        in_maps.append({
            "xin": np.ascontiguousarray(xin[:, m * BC:(m + 1) * BC]),
            "clhs": clhs,
            "elhs": elhs,
            "tlhs": tlhs,
            "sgb": sgb,
        })
    return in_maps


def _get_module():
    global _CACHED_NC
    if _CACHED_NC is None:
        _CACHED_NC = _build_module()
    return _CACHED_NC


class _Runner:
    """Caches the sharded jitted executable so repeat kernel() calls skip
    retracing/compilation (mirrors bass2jax.run_bass_via_pjrt)."""

    def __init__(self, nc):
        import jax
        from jax.sharding import Mesh, PartitionSpec
        try:
            from jax.experimental.shard_map import shard_map
        except ImportError:
            from jax.sharding import shard_map  # newer jax
        from concourse import bass2jax, mybir as mb

        bass2jax.install_neuronx_cc_hook()
        self.jax = jax
        partition_name = (
            nc.partition_id_tensor.name if nc.partition_id_tensor else None
        )
        in_names, out_names, out_avals, zero_shapes = [], [], [], []
        for alloc in nc.m.functions[0].allocations:
            if not isinstance(alloc, mb.MemoryLocationSet):
                continue
            name = alloc.memorylocations[0].name
            if alloc.kind == "ExternalInput":
                if name != partition_name:
                    in_names.append(name)
            elif alloc.kind == "ExternalOutput":
                shape = tuple(alloc.tensor_shape)
                dtype = mb.dt.np(alloc.dtype)
                out_names.append(name)
                out_avals.append(jax.core.ShapedArray(shape, dtype))
                zero_shapes.append((shape, dtype))
        self.in_names, self.out_names = in_names, out_names
        self.out_avals, self.zero_shapes = out_avals, zero_shapes
        n_params, n_outs = len(in_names), len(out_names)
        all_names = in_names + out_names
        if partition_name is not None:
            all_names = all_names + [partition_name]

        def _body(*args):
            operands = list(args)
            if partition_name is not None:
                operands.append(bass2jax.partition_id_tensor())
            outs = bass2jax._bass_exec_p.bind(
                *operands,
                out_avals=tuple(out_avals),
                in_names=tuple(all_names),
                out_names=tuple(out_names),
                lowering_input_output_aliases=(),
                sim_require_finite=True,
                sim_require_nnan=True,
                nc=nc,
            )
            return tuple(outs)

        devices = jax.devices()[:NCORES]
        self.mesh = Mesh(np.asarray(devices), ("core",))
        self.pspec = PartitionSpec("core")
        in_specs = (self.pspec,) * (n_params + n_outs)
        out_specs = (self.pspec,) * n_outs
        self.sharded = jax.jit(
            shard_map(_body, mesh=self.mesh, in_specs=in_specs,
                      out_specs=out_specs, check_rep=False),
            donate_argnums=tuple(range(n_params, n_params + n_outs)),
            keep_unused=True,
        )

    def concat_inputs(self, in_maps):
        return [
            np.concatenate([np.asarray(m[name]) for m in in_maps], axis=0)
            for name in self.in_names
        ]

    def zeros(self):
        return [np.zeros((NCORES * s[0], *s[1:]), d) for s, d in self.zero_shapes]

    def __call__(self, in_maps):
        out_arrs = self.sharded(*self.concat_inputs(in_maps), *self.zeros())
        return [
            {name: np.asarray(out_arrs[i]).reshape(NCORES, *self.out_avals[i].shape)[c]
             for i, name in enumerate(self.out_names)}
            for c in range(NCORES)
        ]


_RUNNERS = {}


def _get_runner(loops=1):
    if loops not in _RUNNERS:
        nc = _get_module() if loops == 1 else _build_module(loops)
        _RUNNERS[loops] = _Runner(nc)
    return _RUNNERS[loops]


def kernel(inp, centroids, radii, W, b):
    in_maps = _prep_inputs(inp, centroids, radii, W, b)
    results = _get_runner()(in_maps)
    return np.concatenate([results[m]["out"] for m in range(NCORES)], axis=0)

